# revision 2
# baseline (speedup 1.0000x reference)
"""Trainium2 Bass kernel for nn_Dilate: 5x5 max-filter (cv2.dilate) over
(64, 384, 384, 3) fp32 images, SAME padding, output (64, 384, 384, 3, 1).

Sharding: pure batch data-parallel, 8 images per NeuronCore.
Per core: [3072 rows, 1152 cols] fp32; partition p owns 24 rows.

v3 = v2 (fp16 DVE compute at 2x, ACT converts, tall in-place wavefront)
plus a split top edge so nothing on the critical path waits for the
top-halo DMAs:

  * Main wavefront covers output rows [2,24) per partition. Its fp16
    tile T holds in rows [0,26) (24 own + 2 bottom halo); the first conv
    chunk needs only the first main DMA => DVE starts ~6us in.
  * Output rows 0,1 (which need the 2 rows above the block, i.e. the
    top halo from the neighbouring partition / zero at image edges) are
    computed in a 6-row side tile T2 (in rows -2..3; rows 0..3
    duplicated from the landing buffer). The top-halo DMAs and the T2
    mini-wavefront run mid-stream, fully overlapped.
  * DMA-in is exactly 24 main rows + 2x15/16 halo rows per partition.
"""

import numpy as np


def _ensure_path():
    try:
        import concourse  # noqa: F401
    except ImportError:
        import sys

        for p in ("/opt/trn_rl_repo", "/root/.axon_site/_ro/trn_rl_repo"):
            if p not in sys.path:
                sys.path.insert(0, p)


N_CORES = 8
B_PER = 8
H = 384
W = 384
C = 3
WROW = W * C  # 1152
ROWS = B_PER * H  # 3072
RP = ROWS // 128  # 24 rows per partition
PAD = 6
PADW = WROW + 2 * PAD  # 1164
NTM = RP + 2  # 26 tile rows in T: in rows [0, 26) (2 bottom halo)

# chunk end indices (T-row space; T row i = in row i rel. to the block)
CONV_E = [2, 4, 7, 10, 14, 18, 21, 23, 25, 26]  # conv T[i] <- L[i+2]
V1_E = [1, 3, 6, 9, 13, 17, 20, 22, 24, 25]  # over [0, 25)
V2_E = [2, 5, 8, 12, 16, 19, 21, 23, 24]  # over [0, 24)
V3_E = [3, 6, 10, 14, 17, 19, 20, 21, 22]  # over [0, 22); out row = i+2
EDGE_AFTER = 3  # insert T2 mini-wave after this main group
TOP_HALO_AFTER = 4  # emit top-halo DMAs after this main chunk
BOT_HALO_AFTER = 7  # emit bottom-halo DMAs after this main chunk
SPLIT_N = 2  # first N main-DMA/conv chunks are emitted as 1-row ops
FUSE_LAST = 1  # last N out chunks: H3 writes fp32 straight into L (no ACT conv)

_CACHE = {}


def _chunks(ends, lo=0):
    out = []
    for e in ends:
        out.append((lo, e))
        lo = e
    return out


def _check_edges(conv_e, v1_e, v2_e, v3_e):
    assert conv_e[-1] == NTM and v1_e[-1] == NTM - 1 and v2_e[-1] == NTM - 2
    assert v3_e[-1] == RP - 2
    assert len(v1_e) == len(conv_e)
    # emission: g0: v1c0; g_k (k>=1): conv ck, v1 ck, v2 c(k-1), v3 c(k-2)...
    for k in range(len(v1_e)):
        assert v1_e[k] + 1 <= conv_e[k], (k, "v1 needs conv rows <= i+1")
    assert len(v2_e) == len(v1_e) - 1
    for k in range(len(v2_e)):
        assert v2_e[k] + 1 <= v1_e[k + 1], (k, "v2 needs v1 rows <= i+1")
    assert len(v3_e) == len(v2_e) - 1 + 1
    for k in range(len(v3_e)):
        assert v3_e[k] + 2 <= v2_e[min(k + 1, len(v2_e) - 1)], (
            k,
            "v3 needs v2 rows <= i+2",
        )


def _build_nc(conv_e=None, v1_e=None, v2_e=None, v3_e=None, edge_after=None):
    _ensure_path()
    from concourse import bacc, mybir, tile
    from concourse.ap import AP

    f32 = mybir.dt.float32
    f16 = mybir.dt.float16

    conv_e = list(conv_e or CONV_E)
    v1_e = list(v1_e or V1_E)
    v2_e = list(v2_e or V2_E)
    v3_e = list(v3_e or V3_E)
    edge_after = EDGE_AFTER if edge_after is None else edge_after
    _check_edges(conv_e, v1_e, v2_e, v3_e)

    nc = bacc.Bacc(
        "TRN2",
        target_bir_lowering=False,
        debug=False,
        enable_asserts=False,
        num_devices=N_CORES,
    )
    x = nc.dram_tensor("x", [ROWS, WROW], f32, kind="ExternalInput")
    y = nc.dram_tensor("y", [ROWS, WROW], f32, kind="ExternalOutput")

    W0 = PAD
    W1 = PAD + WROW

    def xap(row_off, nrows, nparts=128, part0=0):
        return AP(
            x,
            (RP * part0 + row_off) * WROW,
            [[RP * WROW, nparts], [WROW, nrows], [1, WROW]],
        )

    def yap(row_off, nrows):
        return AP(
            y,
            row_off * WROW,
            [[RP * WROW, 128], [WROW, nrows], [1, WROW]],
        )

    conv_c = _chunks(conv_e)
    v1_c = _chunks(v1_e)
    v2_c = _chunks(v2_e)
    v3_c = _chunks(v3_e)
    n = len(conv_e)

    with tile.TileContext(nc) as tc:
        with tc.tile_pool(name="pool", bufs=1) as pool:
            # T row i = in row i (i in [0,26); rows 24,25 = bottom halo)
            T = pool.tile([128, NTM, PADW], f16, name="T", tag="T")
            # T2 row j = in row j-2 (edge tile: 2 top halo + 4 dup rows)
            T2 = pool.tile([128, 6, PADW], f16, name="T2", tag="T2")
            # L row m = in row m-2; [0,2) top halo, [2,26) main,
            # [26,28) bottom halo. Out staging reuses rows [2, 26).
            L = pool.tile([128, RP + 4, WROW], f32, name="L", tag="L")

            e = nc.vector
            s = nc.scalar

            # Setup memzeros run on the DVE (idle during the fill window
            # anyway) so ACT's in-order stream is pure converts — ACT's
            # 4-deep lookahead would otherwise run these ready memzeros
            # ahead of the DMA-blocked first convs.
            def dve_memzero(ap):
                u32 = ap.bitcast(mybir.dt.uint32)
                e.tensor_scalar_mul(u32, u32, 0)

            dve_memzero(T[:, :, 0:PAD])
            dve_memzero(T[:, :, W1:PADW])
            dve_memzero(T2[:, :, 0:PAD])
            dve_memzero(T2[:, :, W1:PADW])
            dve_memzero(L[:, 0:2, :])
            dve_memzero(L[:, RP + 2 : RP + 4, :])
            # dep-free warmup op so the one-time activation-table load
            # (injected before ACT's first Activation) runs at t=0, not
            # when the first conv's input lands
            s.copy(T2[:, 0:1, 0:PAD], T2[:, 0:1, 0:PAD])

            # --- DMA-in: mains in wavefront order; halos mid-stream.
            # conv chunks map to L rows [2,26); the last conv chunk
            # [24,26) reads halo L rows [26,28) and has no main DMA.
            mains = []
            for a, b in conv_c:
                lo, hi = a + 2, min(b + 2, RP + 2)
                if hi > lo:
                    mains.append((lo, hi))

            def main_dma(mi):
                lo, hi = mains[mi]
                if mi < SPLIT_N:
                    # 1-row transfers at the head of the pipe: each conv
                    # row can start as soon as its own row lands
                    for r in range(lo, hi):
                        nc.sync.dma_start(L[:, r : r + 1, :], xap(r - 2, 1))
                else:
                    nc.sync.dma_start(L[:, lo:hi, :], xap(lo - 2, hi - lo))

            def top_halos():
                for b in range(B_PER):
                    p0 = 16 * b + 1
                    nc.sync.dma_start(
                        L[p0 : p0 + 15, 0:2, :], xap(-2, 2, nparts=15, part0=p0)
                    )

            def bottom_halos():
                for b in range(B_PER):
                    p0 = 16 * b
                    nc.sync.dma_start(
                        L[p0 : p0 + 15, RP + 2 : RP + 4, :],
                        xap(RP, 2, nparts=15, part0=p0),
                    )

            for mi in range(len(mains)):
                main_dma(mi)
                if mi == min(TOP_HALO_AFTER, len(mains) - 1):
                    top_halos()
                if mi == min(BOT_HALO_AFTER, len(mains) - 1):
                    bottom_halos()

            # --- compute ops ---
            def conv_in(ci):
                a, b = conv_c[ci]
                if b <= a:
                    return
                if ci < SPLIT_N:
                    for r in range(a, b):
                        s.copy(T[:, r : r + 1, W0:W1], L[:, r + 2 : r + 3, :])
                else:
                    s.copy(T[:, a:b, W0:W1], L[:, a + 2 : b + 2, :])

            def vshift(chunk, shift):
                a, b = chunk
                if b <= a:
                    return
                e.tensor_max(
                    T[:, a:b, W0:W1], T[:, a:b, W0:W1],
                    T[:, a + shift : b + shift, W0:W1],
                )

            def hpasses(tile_, a, b, h3_out=None):
                e.tensor_max(
                    tile_[:, a:b, 0 : PADW - 6], tile_[:, a:b, 0 : PADW - 6],
                    tile_[:, a:b, 6:PADW],
                )
                e.tensor_max(
                    tile_[:, a:b, 0 : PADW - 9], tile_[:, a:b, 0 : PADW - 9],
                    tile_[:, a:b, 3 : PADW - 6],
                )
                # h3_out: write the final pass fp32 straight to the out
                # staging rows (1x rate, but skips the ACT convert hop on
                # the terminal chain)
                out_ap = (
                    tile_[:, a:b, 0:WROW] if h3_out is None else h3_out
                )
                e.tensor_max(
                    out_ap, tile_[:, a:b, 0:WROW],
                    tile_[:, a:b, 3 : 3 + WROW],
                )

            def out_conv(ci):
                # stage at L[a+6:b+6): keeps L[2:6) (edge dup rows) and
                # L[26:28) (bottom halo, read by the last conv chunk at an
                # earlier group) intact until their readers are done.
                a, b = v3_c[ci]
                s.copy(L[:, a + 6 : b + 6, :], T[:, a:b, 0:WROW])

            def out_dma(ci):
                a, b = v3_c[ci]
                nc.sync.dma_start(yap(a + 2, b - a), L[:, a + 6 : b + 6, :])

            def edge_ops():
                # T2 <- L rows [0,6) = in rows -2..3 (halo + 4 dup rows)
                s.copy(T2[:, 0:6, W0:W1], L[:, 0:6, :])
                e.tensor_max(
                    T2[:, 0:5, W0:W1], T2[:, 0:5, W0:W1], T2[:, 1:6, W0:W1]
                )
                e.tensor_max(
                    T2[:, 0:4, W0:W1], T2[:, 0:4, W0:W1], T2[:, 1:5, W0:W1]
                )
                e.tensor_max(
                    T2[:, 0:2, W0:W1], T2[:, 0:2, W0:W1], T2[:, 2:4, W0:W1]
                )
                hpasses(T2, 0, 2)
                s.copy(L[:, 2:4, :], T2[:, 0:2, 0:WROW])
                nc.sync.dma_start(yap(0, 2), L[:, 2:4, :])

            # --- wavefront emission ---
            # group k: conv ck, v1 ck, v2 c(k-1), v3 c(k-2), H+out c(k-2)
            conv_in(0)
            v1c = v2c = v3c = 0
            for k in range(n + 2):
                if k + 1 < n:
                    conv_in(k + 1)
                if v1c <= k and v1c < len(v1_c):
                    vshift(v1_c[v1c], 1)
                    v1c += 1
                if v2c <= k - 1 and v2c < len(v2_c):
                    vshift(v2_c[v2c], 1)
                    v2c += 1
                if v3c <= k - 2 and v3c < len(v3_c):
                    a, b = v3_c[v3c]
                    if b > a:
                        vshift(v3_c[v3c], 2)
                        if v3c >= len(v3_c) - FUSE_LAST:
                            hpasses(T, a, b, h3_out=L[:, a + 6 : b + 6, :])
                        else:
                            hpasses(T, a, b)
                            out_conv(v3c)
                        out_dma(v3c)
                    v3c += 1
                if k == edge_after:
                    edge_ops()

    nc.compile()
    return nc


def _get_nc():
    if "nc" not in _CACHE:
        _CACHE["nc"] = _build_nc()
    return _CACHE["nc"]


def _run(images, trace=False):
    _ensure_path()
    from concourse import bass_utils

    images = np.ascontiguousarray(np.asarray(images, dtype=np.float32))
    assert images.shape == (N_CORES * B_PER, H, W, C), images.shape
    nc = _get_nc()
    per_core = images.reshape(N_CORES, ROWS, WROW)
    in_maps = [{"x": np.ascontiguousarray(per_core[i])} for i in range(N_CORES)]
    res = bass_utils.run_bass_kernel_spmd(
        nc, in_maps, core_ids=list(range(N_CORES)), trace=trace
    )
    out = np.concatenate([res.results[i]["y"] for i in range(N_CORES)], axis=0)
    out = out.reshape(N_CORES * B_PER, H, W, C)[..., None]
    return out, res


def kernel(images, k=None):
    out, _ = _run(images, trace=False)
    return out


# revision 3
# speedup vs baseline: 1.0012x; 1.0012x over previous
"""Trainium2 Bass kernel for nn_Dilate: 5x5 max-filter (cv2.dilate) over
(64, 384, 384, 3) fp32 images, SAME padding, output (64, 384, 384, 3, 1).

Sharding: pure batch data-parallel, 8 images per NeuronCore.
Per core: [3072 rows, 1152 cols] fp32; partition p owns 24 rows.

v3 = v2 (fp16 DVE compute at 2x, ACT converts, tall in-place wavefront)
plus a split top edge so nothing on the critical path waits for the
top-halo DMAs:

  * Main wavefront covers output rows [2,24) per partition. Its fp16
    tile T holds in rows [0,26) (24 own + 2 bottom halo); the first conv
    chunk needs only the first main DMA => DVE starts ~6us in.
  * Output rows 0,1 (which need the 2 rows above the block, i.e. the
    top halo from the neighbouring partition / zero at image edges) are
    computed in a 6-row side tile T2 (in rows -2..3; rows 0..3
    duplicated from the landing buffer). The top-halo DMAs and the T2
    mini-wavefront run mid-stream, fully overlapped.
  * DMA-in is exactly 24 main rows + 2x15/16 halo rows per partition.
"""

import numpy as np


def _ensure_path():
    try:
        import concourse  # noqa: F401
    except ImportError:
        import sys

        for p in ("/opt/trn_rl_repo", "/root/.axon_site/_ro/trn_rl_repo"):
            if p not in sys.path:
                sys.path.insert(0, p)


N_CORES = 8
B_PER = 8
H = 384
W = 384
C = 3
WROW = W * C  # 1152
ROWS = B_PER * H  # 3072
RP = ROWS // 128  # 24 rows per partition
PAD = 6
PADW = WROW + 2 * PAD  # 1164
NTM = RP + 2  # 26 tile rows in T: in rows [0, 26) (2 bottom halo)

# chunk end indices (T-row space; T row i = in row i rel. to the block)
CONV_E = [2, 4, 7, 10, 15, 19, 21, 23, 25, 26]  # conv T[i] <- L[i+2]
V1_E = [1, 3, 6, 9, 14, 18, 20, 22, 24, 25]  # over [0, 25)
V2_E = [2, 5, 8, 13, 17, 19, 21, 23, 24]  # over [0, 24)
V3_E = [3, 6, 11, 15, 17, 19, 20, 21, 22]  # over [0, 22); out row = i+2
EDGE_AFTER = 3  # insert T2 mini-wave after this main group
TOP_HALO_AFTER = 4  # emit top-halo DMAs after this main chunk
BOT_HALO_AFTER = 7  # emit bottom-halo DMAs after this main chunk
SPLIT_N = 2  # first N main-DMA/conv chunks are emitted as 1-row ops
FUSE_LAST = 1  # last N out chunks: H3 writes fp32 straight into L (no ACT conv)

_CACHE = {}


def _chunks(ends, lo=0):
    out = []
    for e in ends:
        out.append((lo, e))
        lo = e
    return out


def _check_edges(conv_e, v1_e, v2_e, v3_e):
    assert conv_e[-1] == NTM and v1_e[-1] == NTM - 1 and v2_e[-1] == NTM - 2
    assert v3_e[-1] == RP - 2
    assert len(v1_e) == len(conv_e)
    # emission: g0: v1c0; g_k (k>=1): conv ck, v1 ck, v2 c(k-1), v3 c(k-2)...
    for k in range(len(v1_e)):
        assert v1_e[k] + 1 <= conv_e[k], (k, "v1 needs conv rows <= i+1")
    assert len(v2_e) == len(v1_e) - 1
    for k in range(len(v2_e)):
        assert v2_e[k] + 1 <= v1_e[k + 1], (k, "v2 needs v1 rows <= i+1")
    assert len(v3_e) == len(v2_e) - 1 + 1
    for k in range(len(v3_e)):
        assert v3_e[k] + 2 <= v2_e[min(k + 1, len(v2_e) - 1)], (
            k,
            "v3 needs v2 rows <= i+2",
        )


def _build_nc(conv_e=None, v1_e=None, v2_e=None, v3_e=None, edge_after=None):
    _ensure_path()
    from concourse import bacc, mybir, tile
    from concourse.ap import AP

    f32 = mybir.dt.float32
    f16 = mybir.dt.float16

    conv_e = list(conv_e or CONV_E)
    v1_e = list(v1_e or V1_E)
    v2_e = list(v2_e or V2_E)
    v3_e = list(v3_e or V3_E)
    edge_after = EDGE_AFTER if edge_after is None else edge_after
    _check_edges(conv_e, v1_e, v2_e, v3_e)

    nc = bacc.Bacc(
        "TRN2",
        target_bir_lowering=False,
        debug=False,
        enable_asserts=False,
        num_devices=N_CORES,
    )
    x = nc.dram_tensor("x", [ROWS, WROW], f32, kind="ExternalInput")
    y = nc.dram_tensor("y", [ROWS, WROW], f32, kind="ExternalOutput")

    W0 = PAD
    W1 = PAD + WROW

    def xap(row_off, nrows, nparts=128, part0=0):
        return AP(
            x,
            (RP * part0 + row_off) * WROW,
            [[RP * WROW, nparts], [WROW, nrows], [1, WROW]],
        )

    def yap(row_off, nrows):
        return AP(
            y,
            row_off * WROW,
            [[RP * WROW, 128], [WROW, nrows], [1, WROW]],
        )

    conv_c = _chunks(conv_e)
    v1_c = _chunks(v1_e)
    v2_c = _chunks(v2_e)
    v3_c = _chunks(v3_e)
    n = len(conv_e)

    with tile.TileContext(nc) as tc:
        with tc.tile_pool(name="pool", bufs=1) as pool:
            # T row i = in row i (i in [0,26); rows 24,25 = bottom halo)
            T = pool.tile([128, NTM, PADW], f16, name="T", tag="T")
            # T2 row j = in row j-2 (edge tile: 2 top halo + 4 dup rows)
            T2 = pool.tile([128, 6, PADW], f16, name="T2", tag="T2")
            # L row m = in row m-2; [0,2) top halo, [2,26) main,
            # [26,28) bottom halo. Out staging reuses rows [2, 26).
            L = pool.tile([128, RP + 4, WROW], f32, name="L", tag="L")

            e = nc.vector
            s = nc.scalar

            # Setup memzeros run on the DVE (idle during the fill window
            # anyway) so ACT's in-order stream is pure converts — ACT's
            # 4-deep lookahead would otherwise run these ready memzeros
            # ahead of the DMA-blocked first convs.
            def dve_memzero(ap):
                u32 = ap.bitcast(mybir.dt.uint32)
                e.tensor_scalar_mul(u32, u32, 0)

            dve_memzero(T[:, :, 0:PAD])
            dve_memzero(T[:, :, W1:PADW])
            dve_memzero(T2[:, :, 0:PAD])
            dve_memzero(T2[:, :, W1:PADW])
            dve_memzero(L[:, 0:2, :])
            dve_memzero(L[:, RP + 2 : RP + 4, :])
            # dep-free warmup op so the one-time activation-table load
            # (injected before ACT's first Activation) runs at t=0, not
            # when the first conv's input lands
            s.copy(T2[:, 0:1, 0:PAD], T2[:, 0:1, 0:PAD])

            # --- DMA-in: mains in wavefront order; halos mid-stream.
            # conv chunks map to L rows [2,26); the last conv chunk
            # [24,26) reads halo L rows [26,28) and has no main DMA.
            mains = []
            for a, b in conv_c:
                lo, hi = a + 2, min(b + 2, RP + 2)
                if hi > lo:
                    mains.append((lo, hi))

            def main_dma(mi):
                lo, hi = mains[mi]
                if mi < SPLIT_N:
                    # 1-row transfers at the head of the pipe: each conv
                    # row can start as soon as its own row lands
                    for r in range(lo, hi):
                        nc.sync.dma_start(L[:, r : r + 1, :], xap(r - 2, 1))
                else:
                    nc.sync.dma_start(L[:, lo:hi, :], xap(lo - 2, hi - lo))

            def top_halos():
                for b in range(B_PER):
                    p0 = 16 * b + 1
                    nc.sync.dma_start(
                        L[p0 : p0 + 15, 0:2, :], xap(-2, 2, nparts=15, part0=p0)
                    )

            def bottom_halos():
                for b in range(B_PER):
                    p0 = 16 * b
                    nc.sync.dma_start(
                        L[p0 : p0 + 15, RP + 2 : RP + 4, :],
                        xap(RP, 2, nparts=15, part0=p0),
                    )

            for mi in range(len(mains)):
                main_dma(mi)
                if mi == min(TOP_HALO_AFTER, len(mains) - 1):
                    top_halos()
                if mi == min(BOT_HALO_AFTER, len(mains) - 1):
                    bottom_halos()

            # --- compute ops ---
            def conv_in(ci):
                a, b = conv_c[ci]
                if b <= a:
                    return
                if ci < SPLIT_N:
                    for r in range(a, b):
                        s.copy(T[:, r : r + 1, W0:W1], L[:, r + 2 : r + 3, :])
                else:
                    s.copy(T[:, a:b, W0:W1], L[:, a + 2 : b + 2, :])

            def vshift(chunk, shift):
                a, b = chunk
                if b <= a:
                    return
                e.tensor_max(
                    T[:, a:b, W0:W1], T[:, a:b, W0:W1],
                    T[:, a + shift : b + shift, W0:W1],
                )

            def hpasses(tile_, a, b, h3_out=None):
                e.tensor_max(
                    tile_[:, a:b, 0 : PADW - 6], tile_[:, a:b, 0 : PADW - 6],
                    tile_[:, a:b, 6:PADW],
                )
                e.tensor_max(
                    tile_[:, a:b, 0 : PADW - 9], tile_[:, a:b, 0 : PADW - 9],
                    tile_[:, a:b, 3 : PADW - 6],
                )
                # h3_out: write the final pass fp32 straight to the out
                # staging rows (1x rate, but skips the ACT convert hop on
                # the terminal chain)
                out_ap = (
                    tile_[:, a:b, 0:WROW] if h3_out is None else h3_out
                )
                e.tensor_max(
                    out_ap, tile_[:, a:b, 0:WROW],
                    tile_[:, a:b, 3 : 3 + WROW],
                )

            def out_conv(ci):
                # stage at L[a+6:b+6): keeps L[2:6) (edge dup rows) and
                # L[26:28) (bottom halo, read by the last conv chunk at an
                # earlier group) intact until their readers are done.
                a, b = v3_c[ci]
                s.copy(L[:, a + 6 : b + 6, :], T[:, a:b, 0:WROW])

            def out_dma(ci):
                a, b = v3_c[ci]
                nc.sync.dma_start(yap(a + 2, b - a), L[:, a + 6 : b + 6, :])

            def edge_ops():
                # T2 <- L rows [0,6) = in rows -2..3 (halo + 4 dup rows)
                s.copy(T2[:, 0:6, W0:W1], L[:, 0:6, :])
                e.tensor_max(
                    T2[:, 0:5, W0:W1], T2[:, 0:5, W0:W1], T2[:, 1:6, W0:W1]
                )
                e.tensor_max(
                    T2[:, 0:4, W0:W1], T2[:, 0:4, W0:W1], T2[:, 1:5, W0:W1]
                )
                e.tensor_max(
                    T2[:, 0:2, W0:W1], T2[:, 0:2, W0:W1], T2[:, 2:4, W0:W1]
                )
                hpasses(T2, 0, 2)
                s.copy(L[:, 2:4, :], T2[:, 0:2, 0:WROW])
                nc.sync.dma_start(yap(0, 2), L[:, 2:4, :])

            # --- wavefront emission ---
            # group k: conv ck, v1 ck, v2 c(k-1), v3 c(k-2), H+out c(k-2)
            conv_in(0)
            v1c = v2c = v3c = 0
            for k in range(n + 2):
                if k + 1 < n:
                    conv_in(k + 1)
                if v1c <= k and v1c < len(v1_c):
                    vshift(v1_c[v1c], 1)
                    v1c += 1
                if v2c <= k - 1 and v2c < len(v2_c):
                    vshift(v2_c[v2c], 1)
                    v2c += 1
                if v3c <= k - 2 and v3c < len(v3_c):
                    a, b = v3_c[v3c]
                    if b > a:
                        vshift(v3_c[v3c], 2)
                        if v3c >= len(v3_c) - FUSE_LAST:
                            hpasses(T, a, b, h3_out=L[:, a + 6 : b + 6, :])
                        else:
                            hpasses(T, a, b)
                            out_conv(v3c)
                        out_dma(v3c)
                    v3c += 1
                if k == edge_after:
                    edge_ops()

    nc.compile()
    return nc


def _get_nc():
    if "nc" not in _CACHE:
        _CACHE["nc"] = _build_nc()
    return _CACHE["nc"]


def _run(images, trace=False):
    _ensure_path()
    from concourse import bass_utils

    images = np.ascontiguousarray(np.asarray(images, dtype=np.float32))
    assert images.shape == (N_CORES * B_PER, H, W, C), images.shape
    nc = _get_nc()
    per_core = images.reshape(N_CORES, ROWS, WROW)
    in_maps = [{"x": np.ascontiguousarray(per_core[i])} for i in range(N_CORES)]
    res = bass_utils.run_bass_kernel_spmd(
        nc, in_maps, core_ids=list(range(N_CORES)), trace=trace
    )
    out = np.concatenate([res.results[i]["y"] for i in range(N_CORES)], axis=0)
    out = out.reshape(N_CORES * B_PER, H, W, C)[..., None]
    return out, res


def kernel(images, k=None):
    out, _ = _run(images, trace=False)
    return out


# revision 4
# speedup vs baseline: 1.0104x; 1.0092x over previous
"""Trainium2 Bass kernel for nn_Dilate: 5x5 max-filter (cv2.dilate) over
(64, 384, 384, 3) fp32 images, SAME padding, output (64, 384, 384, 3, 1).

Sharding: pure batch data-parallel, 8 images per NeuronCore.
Per core: [3072 rows, 1152 cols] fp32; partition p owns 24 rows.

Design (TimelineSim: 117.6us vs 217.6us fp32 baseline):
  * All six shifted-max passes (vertical win5 = shifts +1,+1,+2 rows;
    horizontal win5 = elem shifts +6,+3,+3, i.e. pixel shifts 2,1,1)
    run on the DVE in fp16, which qualifies for the 2x_1p perf mode
    (2 elems/cycle/lane). Inputs are uniform [0,1): fp16 rel err ~5e-4
    << the 2e-2 tolerance. A 5-op shift-max cover of the 5x5 window
    does not exist (exhaustive search), so 6 passes is the compute
    floor; DVE is the bottleneck engine at ~96us busy, just above the
    ~85us DMA floor (in+out fp32 at the model's 360 GB/s).
  * ACT (scalar engine) does all fp32<->fp16 converts, off the DVE
    critical path. Setup memzeros run on the DVE during its idle fill
    window, keeping ACT's in-order stream pure converts; a dep-free
    warmup op absorbs the one-time activation-table load at t=0.
  * One tall fp16 tile T (in rows [0,26) = 24 own + 2 bottom-halo rows
    per partition) is processed as an in-place row wavefront — no
    chunk-halo recompute. The fp32 landing tile L doubles as the
    out-staging buffer (+6 row shift keeps every later reader intact).
  * Output rows 0,1 need the 2 rows above the partition's block (top
    halo). They are computed in a 6-row side tile T2 (rows 0..3 of the
    block duplicated) so the top-halo DMAs and the T2 mini-wavefront
    run mid-stream, fully off the critical path; the main wavefront's
    first conv chunk needs only the first 1-row main DMA.
  * Per-pass chunk edges are staggered (conv > V1 > V2 > V3/H by the
    shift amounts) and emitted in wavefront order; the last H3 writes
    fp32 directly to L, shortening the drain chain. DMA-in is exactly
    24 main rows + 2x(15/16)x2 halo rows per partition.
"""

import numpy as np


def _ensure_path():
    try:
        import concourse  # noqa: F401
    except ImportError:
        import sys

        for p in ("/opt/trn_rl_repo", "/root/.axon_site/_ro/trn_rl_repo"):
            if p not in sys.path:
                sys.path.insert(0, p)


N_CORES = 8
B_PER = 8
H = 384
W = 384
C = 3
WROW = W * C  # 1152
ROWS = B_PER * H  # 3072
RP = ROWS // 128  # 24 rows per partition
PAD = 6
PADW = WROW + 2 * PAD  # 1164
NTM = RP + 2  # 26 tile rows in T: in rows [0, 26) (2 bottom halo)

# chunk end indices (T-row space; T row i = in row i rel. to the block)
CONV_E = [2, 4, 7, 10, 15, 19, 21, 23, 25, 26]  # conv T[i] <- L[i+2]
V1_E = [1, 3, 6, 9, 14, 18, 20, 22, 24, 25]  # over [0, 25)
V2_E = [2, 5, 8, 13, 17, 19, 21, 23, 24]  # over [0, 24)
V3_E = [3, 6, 11, 15, 17, 19, 20, 21, 22]  # over [0, 22); out row = i+2
EDGE_AFTER = 3  # insert T2 mini-wave after this main group
TOP_HALO_AFTER = 4  # emit top-halo DMAs after this main chunk
BOT_HALO_AFTER = 7  # emit bottom-halo DMAs after this main chunk
SPLIT_N = 2  # first N main-DMA/conv chunks are emitted as 1-row ops
FUSE_LAST = 1  # last N out chunks: H3 writes fp32 straight into L (no ACT conv)

_CACHE = {}


def _chunks(ends, lo=0):
    out = []
    for e in ends:
        out.append((lo, e))
        lo = e
    return out


def _check_edges(conv_e, v1_e, v2_e, v3_e):
    assert conv_e[-1] == NTM and v1_e[-1] == NTM - 1 and v2_e[-1] == NTM - 2
    assert v3_e[-1] == RP - 2
    assert len(v1_e) == len(conv_e)
    # emission: g0: v1c0; g_k (k>=1): conv ck, v1 ck, v2 c(k-1), v3 c(k-2)...
    for k in range(len(v1_e)):
        assert v1_e[k] + 1 <= conv_e[k], (k, "v1 needs conv rows <= i+1")
    assert len(v2_e) == len(v1_e) - 1
    for k in range(len(v2_e)):
        assert v2_e[k] + 1 <= v1_e[k + 1], (k, "v2 needs v1 rows <= i+1")
    assert len(v3_e) == len(v2_e) - 1 + 1
    for k in range(len(v3_e)):
        assert v3_e[k] + 2 <= v2_e[min(k + 1, len(v2_e) - 1)], (
            k,
            "v3 needs v2 rows <= i+2",
        )


def _build_nc(conv_e=None, v1_e=None, v2_e=None, v3_e=None, edge_after=None):
    _ensure_path()
    from concourse import bacc, mybir, tile
    from concourse.ap import AP

    f32 = mybir.dt.float32
    f16 = mybir.dt.float16

    conv_e = list(conv_e or CONV_E)
    v1_e = list(v1_e or V1_E)
    v2_e = list(v2_e or V2_E)
    v3_e = list(v3_e or V3_E)
    edge_after = EDGE_AFTER if edge_after is None else edge_after
    _check_edges(conv_e, v1_e, v2_e, v3_e)

    nc = bacc.Bacc(
        "TRN2",
        target_bir_lowering=False,
        debug=False,
        enable_asserts=False,
        num_devices=N_CORES,
    )
    x = nc.dram_tensor("x", [ROWS, WROW], f32, kind="ExternalInput")
    y = nc.dram_tensor("y", [ROWS, WROW], f32, kind="ExternalOutput")

    W0 = PAD
    W1 = PAD + WROW

    def xap(row_off, nrows, nparts=128, part0=0):
        return AP(
            x,
            (RP * part0 + row_off) * WROW,
            [[RP * WROW, nparts], [WROW, nrows], [1, WROW]],
        )

    def yap(row_off, nrows):
        return AP(
            y,
            row_off * WROW,
            [[RP * WROW, 128], [WROW, nrows], [1, WROW]],
        )

    conv_c = _chunks(conv_e)
    v1_c = _chunks(v1_e)
    v2_c = _chunks(v2_e)
    v3_c = _chunks(v3_e)
    n = len(conv_e)

    with tile.TileContext(nc) as tc:
        with tc.tile_pool(name="pool", bufs=1) as pool:
            # T row i = in row i (i in [0,26); rows 24,25 = bottom halo)
            T = pool.tile([128, NTM, PADW], f16, name="T", tag="T")
            # T2 row j = in row j-2 (edge tile: 2 top halo + 4 dup rows)
            T2 = pool.tile([128, 6, PADW], f16, name="T2", tag="T2")
            # L row m = in row m-2; [0,2) top halo, [2,26) main,
            # [26,28) bottom halo. Out staging reuses rows [2, 26).
            L = pool.tile([128, RP + 4, WROW], f32, name="L", tag="L")

            e = nc.vector
            s = nc.scalar

            # Setup memzeros run on the DVE (idle during the fill window
            # anyway) so ACT's in-order stream is pure converts — ACT's
            # 4-deep lookahead would otherwise run these ready memzeros
            # ahead of the DMA-blocked first convs.
            def dve_memzero(ap):
                u32 = ap.bitcast(mybir.dt.uint32)
                e.tensor_scalar_mul(u32, u32, 0)

            dve_memzero(T[:, :, 0:PAD])
            dve_memzero(T[:, :, W1:PADW])
            dve_memzero(T2[:, :, 0:PAD])
            dve_memzero(T2[:, :, W1:PADW])
            dve_memzero(L[:, 0:2, :])
            dve_memzero(L[:, RP + 2 : RP + 4, :])
            # dep-free warmup op so the one-time activation-table load
            # (injected before ACT's first Activation) runs at t=0, not
            # when the first conv's input lands
            s.copy(T2[:, 0:1, 0:PAD], T2[:, 0:1, 0:PAD])

            # --- DMA-in: mains in wavefront order; halos mid-stream.
            # conv chunks map to L rows [2,26); the last conv chunk
            # [24,26) reads halo L rows [26,28) and has no main DMA.
            mains = []
            for a, b in conv_c:
                lo, hi = a + 2, min(b + 2, RP + 2)
                if hi > lo:
                    mains.append((lo, hi))

            def main_dma(mi):
                lo, hi = mains[mi]
                if mi < SPLIT_N:
                    # 1-row transfers at the head of the pipe: each conv
                    # row can start as soon as its own row lands
                    for r in range(lo, hi):
                        nc.sync.dma_start(L[:, r : r + 1, :], xap(r - 2, 1))
                else:
                    nc.sync.dma_start(L[:, lo:hi, :], xap(lo - 2, hi - lo))

            def top_halos():
                for b in range(B_PER):
                    p0 = 16 * b + 1
                    nc.sync.dma_start(
                        L[p0 : p0 + 15, 0:2, :], xap(-2, 2, nparts=15, part0=p0)
                    )

            def bottom_halos():
                for b in range(B_PER):
                    p0 = 16 * b
                    nc.sync.dma_start(
                        L[p0 : p0 + 15, RP + 2 : RP + 4, :],
                        xap(RP, 2, nparts=15, part0=p0),
                    )

            for mi in range(len(mains)):
                main_dma(mi)
                if mi == min(TOP_HALO_AFTER, len(mains) - 1):
                    top_halos()
                if mi == min(BOT_HALO_AFTER, len(mains) - 1):
                    bottom_halos()

            # --- compute ops ---
            def conv_in(ci):
                a, b = conv_c[ci]
                if b <= a:
                    return
                if ci < SPLIT_N:
                    for r in range(a, b):
                        s.copy(T[:, r : r + 1, W0:W1], L[:, r + 2 : r + 3, :])
                else:
                    s.copy(T[:, a:b, W0:W1], L[:, a + 2 : b + 2, :])

            def vshift(chunk, shift):
                a, b = chunk
                if b <= a:
                    return
                e.tensor_max(
                    T[:, a:b, W0:W1], T[:, a:b, W0:W1],
                    T[:, a + shift : b + shift, W0:W1],
                )

            def hpasses(tile_, a, b, h3_out=None):
                e.tensor_max(
                    tile_[:, a:b, 0 : PADW - 6], tile_[:, a:b, 0 : PADW - 6],
                    tile_[:, a:b, 6:PADW],
                )
                e.tensor_max(
                    tile_[:, a:b, 0 : PADW - 9], tile_[:, a:b, 0 : PADW - 9],
                    tile_[:, a:b, 3 : PADW - 6],
                )
                # h3_out: write the final pass fp32 straight to the out
                # staging rows (1x rate, but skips the ACT convert hop on
                # the terminal chain)
                out_ap = (
                    tile_[:, a:b, 0:WROW] if h3_out is None else h3_out
                )
                e.tensor_max(
                    out_ap, tile_[:, a:b, 0:WROW],
                    tile_[:, a:b, 3 : 3 + WROW],
                )

            def out_conv(ci):
                # stage at L[a+6:b+6): keeps L[2:6) (edge dup rows) and
                # L[26:28) (bottom halo, read by the last conv chunk at an
                # earlier group) intact until their readers are done.
                a, b = v3_c[ci]
                s.copy(L[:, a + 6 : b + 6, :], T[:, a:b, 0:WROW])

            def out_dma(ci):
                a, b = v3_c[ci]
                nc.sync.dma_start(yap(a + 2, b - a), L[:, a + 6 : b + 6, :])

            def edge_ops():
                # T2 <- L rows [0,6) = in rows -2..3 (halo + 4 dup rows)
                s.copy(T2[:, 0:6, W0:W1], L[:, 0:6, :])
                e.tensor_max(
                    T2[:, 0:5, W0:W1], T2[:, 0:5, W0:W1], T2[:, 1:6, W0:W1]
                )
                e.tensor_max(
                    T2[:, 0:4, W0:W1], T2[:, 0:4, W0:W1], T2[:, 1:5, W0:W1]
                )
                e.tensor_max(
                    T2[:, 0:2, W0:W1], T2[:, 0:2, W0:W1], T2[:, 2:4, W0:W1]
                )
                hpasses(T2, 0, 2)
                s.copy(L[:, 2:4, :], T2[:, 0:2, 0:WROW])
                nc.sync.dma_start(yap(0, 2), L[:, 2:4, :])

            # --- wavefront emission ---
            # group k: conv ck, v1 ck, v2 c(k-1), v3 c(k-2), H+out c(k-2)
            conv_in(0)
            v1c = v2c = v3c = 0
            for k in range(n + 2):
                if k + 1 < n:
                    conv_in(k + 1)
                if v1c <= k and v1c < len(v1_c):
                    vshift(v1_c[v1c], 1)
                    v1c += 1
                if v2c <= k - 1 and v2c < len(v2_c):
                    vshift(v2_c[v2c], 1)
                    v2c += 1
                if v3c <= k - 2 and v3c < len(v3_c):
                    a, b = v3_c[v3c]
                    if b > a:
                        vshift(v3_c[v3c], 2)
                        if v3c >= len(v3_c) - FUSE_LAST:
                            hpasses(T, a, b, h3_out=L[:, a + 6 : b + 6, :])
                        else:
                            hpasses(T, a, b)
                            out_conv(v3c)
                        out_dma(v3c)
                    v3c += 1
                if k == edge_after:
                    edge_ops()

    nc.compile()
    return nc


def _get_nc():
    if "nc" not in _CACHE:
        _CACHE["nc"] = _build_nc()
    return _CACHE["nc"]


def _run(images, trace=False):
    _ensure_path()
    from concourse import bass_utils

    images = np.ascontiguousarray(np.asarray(images, dtype=np.float32))
    assert images.shape == (N_CORES * B_PER, H, W, C), images.shape
    nc = _get_nc()
    per_core = images.reshape(N_CORES, ROWS, WROW)
    in_maps = [{"x": np.ascontiguousarray(per_core[i])} for i in range(N_CORES)]
    res = bass_utils.run_bass_kernel_spmd(
        nc, in_maps, core_ids=list(range(N_CORES)), trace=trace
    )
    out = np.concatenate([res.results[i]["y"] for i in range(N_CORES)], axis=0)
    out = out.reshape(N_CORES * B_PER, H, W, C)[..., None]
    return out, res


def kernel(images, k=None):
    out, _ = _run(images, trace=False)
    return out


# revision 5
# speedup vs baseline: 1.0223x; 1.0118x over previous
"""Trainium2 Bass kernel for nn_Dilate: 5x5 max-filter (cv2.dilate) over
(64, 384, 384, 3) fp32 images, SAME padding, output (64, 384, 384, 3, 1).

Sharding: pure batch data-parallel, 8 images per NeuronCore.
Per core: [3072 rows, 1152 cols] fp32; partition p owns 24 rows.

Design (TimelineSim: 117.6us vs 217.6us fp32 baseline):
  * All six shifted-max passes (vertical win5 = shifts +1,+1,+2 rows;
    horizontal win5 = elem shifts +6,+3,+3, i.e. pixel shifts 2,1,1)
    run on the DVE in fp16, which qualifies for the 2x_1p perf mode
    (2 elems/cycle/lane). Inputs are uniform [0,1): fp16 rel err ~5e-4
    << the 2e-2 tolerance. A 5-op shift-max cover of the 5x5 window
    does not exist (exhaustive search), so 6 passes is the compute
    floor; DVE is the bottleneck engine at ~96us busy, just above the
    ~85us DMA floor (in+out fp32 at the model's 360 GB/s).
  * ACT (scalar engine) does all fp32<->fp16 converts, off the DVE
    critical path. Setup memzeros run on the DVE during its idle fill
    window, keeping ACT's in-order stream pure converts; a dep-free
    warmup op absorbs the one-time activation-table load at t=0.
  * One tall fp16 tile T (in rows [0,26) = 24 own + 2 bottom-halo rows
    per partition) is processed as an in-place row wavefront — no
    chunk-halo recompute. The fp32 landing tile L doubles as the
    out-staging buffer (+6 row shift keeps every later reader intact).
  * Output rows 0,1 need the 2 rows above the partition's block (top
    halo). They are computed in a 6-row side tile T2 (rows 0..3 of the
    block duplicated) so the top-halo DMAs and the T2 mini-wavefront
    run mid-stream, fully off the critical path; the main wavefront's
    first conv chunk needs only the first 1-row main DMA.
  * Per-pass chunk edges are staggered (conv > V1 > V2 > V3/H by the
    shift amounts) and emitted in wavefront order; the last H3 writes
    fp32 directly to L, shortening the drain chain. DMA-in is exactly
    24 main rows + 2x(15/16)x2 halo rows per partition.
"""

import numpy as np


def _ensure_path():
    try:
        import concourse  # noqa: F401
    except ImportError:
        import sys

        for p in ("/opt/trn_rl_repo", "/root/.axon_site/_ro/trn_rl_repo"):
            if p not in sys.path:
                sys.path.insert(0, p)


N_CORES = 8
B_PER = 8
H = 384
W = 384
C = 3
WROW = W * C  # 1152
ROWS = B_PER * H  # 3072
RP = ROWS // 128  # 24 rows per partition
PAD = 6
PADW = WROW + 2 * PAD  # 1164
NTM = RP + 2  # 26 tile rows in T: in rows [0, 26) (2 bottom halo)

# chunk end indices (T-row space; T row i = in row i rel. to the block)
CONV_E = [2, 4, 7, 10, 15, 19, 21, 23, 25, 26]  # conv T[i] <- L[i+2]
V1_E = [1, 3, 6, 9, 14, 18, 20, 22, 24, 25]  # over [0, 25)
V2_E = [2, 5, 8, 13, 17, 19, 21, 23, 24]  # over [0, 24)
V3_E = [3, 6, 11, 15, 17, 19, 20, 21, 22]  # over [0, 22); out row = i+2
EDGE_AFTER = 3  # insert T2 mini-wave after this main group
TOP_HALO_AFTER = 4  # emit top-halo DMAs after this main chunk
BOT_HALO_AFTER = 7  # emit bottom-halo DMAs after this main chunk
SPLIT_N = 2  # first N main-DMA/conv chunks are emitted as 1-row ops
FUSE_LAST = 1  # last N out chunks: H3 writes fp32 straight into L (no ACT conv)
DVE_CONV_N = 3  # first N conv chunks converted on the DVE (fills its fill-window
# idle, skips a cross-engine hop, and lets ACT start at conv chunk N)
TAIL_COLSPLIT = True  # column-split the final row's fused H3 + out-DMA

_CACHE = {}


def _chunks(ends, lo=0):
    out = []
    for e in ends:
        out.append((lo, e))
        lo = e
    return out


def _check_edges(conv_e, v1_e, v2_e, v3_e):
    assert conv_e[-1] == NTM and v1_e[-1] == NTM - 1 and v2_e[-1] == NTM - 2
    assert v3_e[-1] == RP - 2
    assert len(v1_e) == len(conv_e)
    # emission: g0: v1c0; g_k (k>=1): conv ck, v1 ck, v2 c(k-1), v3 c(k-2)...
    for k in range(len(v1_e)):
        assert v1_e[k] + 1 <= conv_e[k], (k, "v1 needs conv rows <= i+1")
    assert len(v2_e) == len(v1_e) - 1
    for k in range(len(v2_e)):
        assert v2_e[k] + 1 <= v1_e[k + 1], (k, "v2 needs v1 rows <= i+1")
    assert len(v3_e) == len(v2_e) - 1 + 1
    for k in range(len(v3_e)):
        assert v3_e[k] + 2 <= v2_e[min(k + 1, len(v2_e) - 1)], (
            k,
            "v3 needs v2 rows <= i+2",
        )


def _build_nc(conv_e=None, v1_e=None, v2_e=None, v3_e=None, edge_after=None):
    _ensure_path()
    from concourse import bacc, mybir, tile
    from concourse.ap import AP

    f32 = mybir.dt.float32
    f16 = mybir.dt.float16

    conv_e = list(conv_e or CONV_E)
    v1_e = list(v1_e or V1_E)
    v2_e = list(v2_e or V2_E)
    v3_e = list(v3_e or V3_E)
    edge_after = EDGE_AFTER if edge_after is None else edge_after
    _check_edges(conv_e, v1_e, v2_e, v3_e)

    nc = bacc.Bacc(
        "TRN2",
        target_bir_lowering=False,
        debug=False,
        enable_asserts=False,
        num_devices=N_CORES,
    )
    x = nc.dram_tensor("x", [ROWS, WROW], f32, kind="ExternalInput")
    y = nc.dram_tensor("y", [ROWS, WROW], f32, kind="ExternalOutput")

    W0 = PAD
    W1 = PAD + WROW

    def xap(row_off, nrows, nparts=128, part0=0):
        return AP(
            x,
            (RP * part0 + row_off) * WROW,
            [[RP * WROW, nparts], [WROW, nrows], [1, WROW]],
        )

    def yap(row_off, nrows):
        return AP(
            y,
            row_off * WROW,
            [[RP * WROW, 128], [WROW, nrows], [1, WROW]],
        )

    conv_c = _chunks(conv_e)
    v1_c = _chunks(v1_e)
    v2_c = _chunks(v2_e)
    v3_c = _chunks(v3_e)
    n = len(conv_e)

    with tile.TileContext(nc) as tc:
        with tc.tile_pool(name="pool", bufs=1) as pool:
            # T row i = in row i (i in [0,26); rows 24,25 = bottom halo)
            T = pool.tile([128, NTM, PADW], f16, name="T", tag="T")
            # T2 row j = in row j-2 (edge tile: 2 top halo + 4 dup rows)
            T2 = pool.tile([128, 6, PADW], f16, name="T2", tag="T2")
            # L row m = in row m-2; [0,2) top halo, [2,26) main,
            # [26,28) bottom halo. Out staging reuses rows [2, 26).
            L = pool.tile([128, RP + 4, WROW], f32, name="L", tag="L")

            e = nc.vector
            s = nc.scalar

            # Setup memzeros run on the DVE (idle during the fill window
            # anyway) so ACT's in-order stream is pure converts — ACT's
            # 4-deep lookahead would otherwise run these ready memzeros
            # ahead of the DMA-blocked first convs.
            def dve_memzero(ap):
                u32 = ap.bitcast(mybir.dt.uint32)
                e.tensor_scalar_mul(u32, u32, 0)

            dve_memzero(T[:, :, 0:PAD])
            dve_memzero(T[:, :, W1:PADW])
            dve_memzero(T2[:, :, 0:PAD])
            dve_memzero(T2[:, :, W1:PADW])
            dve_memzero(L[:, 0:2, :])
            dve_memzero(L[:, RP + 2 : RP + 4, :])
            # dep-free warmup op so the one-time activation-table load
            # (injected before ACT's first Activation) runs at t=0, not
            # when the first conv's input lands
            s.copy(T2[:, 0:1, 0:PAD], T2[:, 0:1, 0:PAD])

            # --- DMA-in: mains in wavefront order; halos mid-stream.
            # conv chunks map to L rows [2,26); the last conv chunk
            # [24,26) reads halo L rows [26,28) and has no main DMA.
            mains = []
            for a, b in conv_c:
                lo, hi = a + 2, min(b + 2, RP + 2)
                if hi > lo:
                    mains.append((lo, hi))

            def main_dma(mi):
                lo, hi = mains[mi]
                if mi < SPLIT_N:
                    # 1-row transfers at the head of the pipe: each conv
                    # row can start as soon as its own row lands
                    for r in range(lo, hi):
                        nc.sync.dma_start(L[:, r : r + 1, :], xap(r - 2, 1))
                else:
                    nc.sync.dma_start(L[:, lo:hi, :], xap(lo - 2, hi - lo))

            def top_halos():
                for b in range(B_PER):
                    p0 = 16 * b + 1
                    nc.sync.dma_start(
                        L[p0 : p0 + 15, 0:2, :], xap(-2, 2, nparts=15, part0=p0)
                    )

            def bottom_halos():
                for b in range(B_PER):
                    p0 = 16 * b
                    nc.sync.dma_start(
                        L[p0 : p0 + 15, RP + 2 : RP + 4, :],
                        xap(RP, 2, nparts=15, part0=p0),
                    )

            for mi in range(len(mains)):
                main_dma(mi)
                if mi == min(TOP_HALO_AFTER, len(mains) - 1):
                    top_halos()
                if mi == min(BOT_HALO_AFTER, len(mains) - 1):
                    bottom_halos()

            # --- compute ops ---
            def conv_in(ci):
                a, b = conv_c[ci]
                if b <= a:
                    return
                if ci < SPLIT_N or ci < DVE_CONV_N:
                    for r in range(a, b):
                        eng = e if ci < DVE_CONV_N else s
                        if eng is e:
                            e.tensor_copy(
                                T[:, r : r + 1, W0:W1], L[:, r + 2 : r + 3, :]
                            )
                        else:
                            s.copy(T[:, r : r + 1, W0:W1], L[:, r + 2 : r + 3, :])
                else:
                    s.copy(T[:, a:b, W0:W1], L[:, a + 2 : b + 2, :])

            def vshift(chunk, shift):
                a, b = chunk
                if b <= a:
                    return
                e.tensor_max(
                    T[:, a:b, W0:W1], T[:, a:b, W0:W1],
                    T[:, a + shift : b + shift, W0:W1],
                )

            def hpasses(tile_, a, b, h3_out=None):
                e.tensor_max(
                    tile_[:, a:b, 0 : PADW - 6], tile_[:, a:b, 0 : PADW - 6],
                    tile_[:, a:b, 6:PADW],
                )
                e.tensor_max(
                    tile_[:, a:b, 0 : PADW - 9], tile_[:, a:b, 0 : PADW - 9],
                    tile_[:, a:b, 3 : PADW - 6],
                )
                # h3_out: write the final pass fp32 straight to the out
                # staging rows (1x rate, but skips the ACT convert hop on
                # the terminal chain)
                out_ap = (
                    tile_[:, a:b, 0:WROW] if h3_out is None else h3_out
                )
                e.tensor_max(
                    out_ap, tile_[:, a:b, 0:WROW],
                    tile_[:, a:b, 3 : 3 + WROW],
                )

            def out_conv(ci):
                # stage at L[a+6:b+6): keeps L[2:6) (edge dup rows) and
                # L[26:28) (bottom halo, read by the last conv chunk at an
                # earlier group) intact until their readers are done.
                a, b = v3_c[ci]
                s.copy(L[:, a + 6 : b + 6, :], T[:, a:b, 0:WROW])

            def out_dma(ci):
                a, b = v3_c[ci]
                nc.sync.dma_start(yap(a + 2, b - a), L[:, a + 6 : b + 6, :])

            def edge_ops():
                # T2 <- L rows [0,6) = in rows -2..3 (halo + 4 dup rows)
                s.copy(T2[:, 0:6, W0:W1], L[:, 0:6, :])
                e.tensor_max(
                    T2[:, 0:5, W0:W1], T2[:, 0:5, W0:W1], T2[:, 1:6, W0:W1]
                )
                e.tensor_max(
                    T2[:, 0:4, W0:W1], T2[:, 0:4, W0:W1], T2[:, 1:5, W0:W1]
                )
                e.tensor_max(
                    T2[:, 0:2, W0:W1], T2[:, 0:2, W0:W1], T2[:, 2:4, W0:W1]
                )
                hpasses(T2, 0, 2)
                s.copy(L[:, 2:4, :], T2[:, 0:2, 0:WROW])
                nc.sync.dma_start(yap(0, 2), L[:, 2:4, :])

            # --- wavefront emission ---
            # group k: conv ck, v1 ck, v2 c(k-1), v3 c(k-2), H+out c(k-2)
            conv_in(0)
            v1c = v2c = v3c = 0
            for k in range(n + 2):
                if k + 1 < n:
                    conv_in(k + 1)
                if v1c <= k and v1c < len(v1_c):
                    vshift(v1_c[v1c], 1)
                    v1c += 1
                if v2c <= k - 1 and v2c < len(v2_c):
                    vshift(v2_c[v2c], 1)
                    v2c += 1
                if v3c <= k - 2 and v3c < len(v3_c):
                    a, b = v3_c[v3c]
                    if b > a:
                        vshift(v3_c[v3c], 2)
                        if v3c == len(v3_c) - 1 and TAIL_COLSPLIT:
                            # final row: H1/H2 whole, then column-split the
                            # fused H3 so each half's out-DMA overlaps the
                            # other half's compute
                            e.tensor_max(
                                T[:, a:b, 0 : PADW - 6],
                                T[:, a:b, 0 : PADW - 6], T[:, a:b, 6:PADW],
                            )
                            e.tensor_max(
                                T[:, a:b, 0 : PADW - 9],
                                T[:, a:b, 0 : PADW - 9],
                                T[:, a:b, 3 : PADW - 6],
                            )
                            hw_ = WROW // 2
                            for c0, c1 in ((0, hw_), (hw_, WROW)):
                                e.tensor_max(
                                    L[:, a + 6 : b + 6, c0:c1],
                                    T[:, a:b, c0:c1],
                                    T[:, a:b, c0 + 3 : c1 + 3],
                                )
                                nc.sync.dma_start(
                                    AP(
                                        y,
                                        (a + 2) * WROW + c0,
                                        [
                                            [RP * WROW, 128],
                                            [WROW, b - a],
                                            [1, c1 - c0],
                                        ],
                                    ),
                                    L[:, a + 6 : b + 6, c0:c1],
                                )
                        elif v3c >= len(v3_c) - FUSE_LAST:
                            hpasses(T, a, b, h3_out=L[:, a + 6 : b + 6, :])
                            out_dma(v3c)
                        else:
                            hpasses(T, a, b)
                            out_conv(v3c)
                            out_dma(v3c)
                    v3c += 1
                if k == edge_after:
                    edge_ops()

    nc.compile()
    return nc


def _get_nc():
    if "nc" not in _CACHE:
        _CACHE["nc"] = _build_nc()
    return _CACHE["nc"]


def _run(images, trace=False):
    _ensure_path()
    from concourse import bass_utils

    images = np.ascontiguousarray(np.asarray(images, dtype=np.float32))
    assert images.shape == (N_CORES * B_PER, H, W, C), images.shape
    nc = _get_nc()
    per_core = images.reshape(N_CORES, ROWS, WROW)
    in_maps = [{"x": np.ascontiguousarray(per_core[i])} for i in range(N_CORES)]
    res = bass_utils.run_bass_kernel_spmd(
        nc, in_maps, core_ids=list(range(N_CORES)), trace=trace
    )
    out = np.concatenate([res.results[i]["y"] for i in range(N_CORES)], axis=0)
    out = out.reshape(N_CORES * B_PER, H, W, C)[..., None]
    return out, res


def kernel(images, k=None):
    out, _ = _run(images, trace=False)
    return out


# revision 6
# speedup vs baseline: 1.0428x; 1.0200x over previous
"""Trainium2 Bass kernel for nn_Dilate: 5x5 max-filter (cv2.dilate) over
(64, 384, 384, 3) fp32 images, SAME padding, output (64, 384, 384, 3, 1).

Sharding: pure batch data-parallel, 8 images per NeuronCore.
Per core: [3072 rows, 1152 cols] fp32; partition p owns 24 rows.

Design (TimelineSim: 117.6us vs 217.6us fp32 baseline):
  * All six shifted-max passes (vertical win5 = shifts +1,+1,+2 rows;
    horizontal win5 = elem shifts +6,+3,+3, i.e. pixel shifts 2,1,1)
    run on the DVE in fp16, which qualifies for the 2x_1p perf mode
    (2 elems/cycle/lane). Inputs are uniform [0,1): fp16 rel err ~5e-4
    << the 2e-2 tolerance. A 5-op shift-max cover of the 5x5 window
    does not exist (exhaustive search), so 6 passes is the compute
    floor; DVE is the bottleneck engine at ~96us busy, just above the
    ~85us DMA floor (in+out fp32 at the model's 360 GB/s).
  * ACT (scalar engine) does all fp32<->fp16 converts, off the DVE
    critical path. Setup memzeros run on the DVE during its idle fill
    window, keeping ACT's in-order stream pure converts; a dep-free
    warmup op absorbs the one-time activation-table load at t=0.
  * One tall fp16 tile T (in rows [0,26) = 24 own + 2 bottom-halo rows
    per partition) is processed as an in-place row wavefront — no
    chunk-halo recompute. The fp32 landing tile L doubles as the
    out-staging buffer (+6 row shift keeps every later reader intact).
  * Output rows 0,1 need the 2 rows above the partition's block (top
    halo). They are computed in a 6-row side tile T2 (rows 0..3 of the
    block duplicated) so the top-halo DMAs and the T2 mini-wavefront
    run mid-stream, fully off the critical path; the main wavefront's
    first conv chunk needs only the first 1-row main DMA.
  * Per-pass chunk edges are staggered (conv > V1 > V2 > V3/H by the
    shift amounts) and emitted in wavefront order; the last H3 writes
    fp32 directly to L, shortening the drain chain. DMA-in is exactly
    24 main rows + 2x(15/16)x2 halo rows per partition.
"""

import numpy as np


def _ensure_path():
    try:
        import concourse  # noqa: F401
    except ImportError:
        import sys

        for p in ("/opt/trn_rl_repo", "/root/.axon_site/_ro/trn_rl_repo"):
            if p not in sys.path:
                sys.path.insert(0, p)


N_CORES = 8
B_PER = 8
H = 384
W = 384
C = 3
WROW = W * C  # 1152
ROWS = B_PER * H  # 3072
RP = ROWS // 128  # 24 rows per partition
PAD = 6
PADW = WROW + 2 * PAD  # 1164
NTM = RP + 2  # 26 tile rows in T: in rows [0, 26) (2 bottom halo)

# chunk end indices (T-row space; T row i = in row i rel. to the block)
CONV_E = [2, 4, 7, 10, 15, 19, 21, 23, 25, 26]  # conv T[i] <- L[i+2]
V1_E = [1, 3, 6, 9, 14, 18, 20, 22, 24, 25]  # over [0, 25)
V2_E = [2, 5, 8, 13, 17, 19, 21, 23, 24]  # over [0, 24)
V3_E = [3, 6, 11, 15, 17, 19, 20, 21, 22]  # over [0, 22); out row = i+2
EDGE_AFTER = 3  # insert T2 mini-wave after this main group
TOP_HALO_AFTER = 5  # emit top-halo DMAs after this main chunk
BOT_HALO_AFTER = 7  # emit bottom-halo DMAs after this main chunk
SPLIT_N = 2  # first N main-DMA/conv chunks are emitted as 1-row ops
FUSE_LAST = 1  # last N out chunks: H3 writes fp32 straight into L (no ACT conv)
DVE_CONV_N = 3  # first N conv chunks converted on the DVE (fills its fill-window
# idle, skips a cross-engine hop, and lets ACT start at conv chunk N)
TAIL_COLSPLIT = True  # column-split the final row's fused H3 + out-DMA

_CACHE = {}


def _chunks(ends, lo=0):
    out = []
    for e in ends:
        out.append((lo, e))
        lo = e
    return out


def _check_edges(conv_e, v1_e, v2_e, v3_e):
    assert conv_e[-1] == NTM and v1_e[-1] == NTM - 1 and v2_e[-1] == NTM - 2
    assert v3_e[-1] == RP - 2
    assert len(v1_e) == len(conv_e)
    # emission: g0: v1c0; g_k (k>=1): conv ck, v1 ck, v2 c(k-1), v3 c(k-2)...
    for k in range(len(v1_e)):
        assert v1_e[k] + 1 <= conv_e[k], (k, "v1 needs conv rows <= i+1")
    assert len(v2_e) == len(v1_e) - 1
    for k in range(len(v2_e)):
        assert v2_e[k] + 1 <= v1_e[k + 1], (k, "v2 needs v1 rows <= i+1")
    assert len(v3_e) == len(v2_e) - 1 + 1
    for k in range(len(v3_e)):
        assert v3_e[k] + 2 <= v2_e[min(k + 1, len(v2_e) - 1)], (
            k,
            "v3 needs v2 rows <= i+2",
        )


def _build_nc(conv_e=None, v1_e=None, v2_e=None, v3_e=None, edge_after=None):
    _ensure_path()
    from concourse import bacc, mybir, tile
    from concourse.ap import AP

    f32 = mybir.dt.float32
    f16 = mybir.dt.float16

    conv_e = list(conv_e or CONV_E)
    v1_e = list(v1_e or V1_E)
    v2_e = list(v2_e or V2_E)
    v3_e = list(v3_e or V3_E)
    edge_after = EDGE_AFTER if edge_after is None else edge_after
    _check_edges(conv_e, v1_e, v2_e, v3_e)

    nc = bacc.Bacc(
        "TRN2",
        target_bir_lowering=False,
        debug=False,
        enable_asserts=False,
        num_devices=N_CORES,
    )
    x = nc.dram_tensor("x", [ROWS, WROW], f32, kind="ExternalInput")
    y = nc.dram_tensor("y", [ROWS, WROW], f32, kind="ExternalOutput")

    W0 = PAD
    W1 = PAD + WROW

    def xap(row_off, nrows, nparts=128, part0=0):
        return AP(
            x,
            (RP * part0 + row_off) * WROW,
            [[RP * WROW, nparts], [WROW, nrows], [1, WROW]],
        )

    def yap(row_off, nrows):
        return AP(
            y,
            row_off * WROW,
            [[RP * WROW, 128], [WROW, nrows], [1, WROW]],
        )

    conv_c = _chunks(conv_e)
    v1_c = _chunks(v1_e)
    v2_c = _chunks(v2_e)
    v3_c = _chunks(v3_e)
    n = len(conv_e)

    with tile.TileContext(nc) as tc:
        with tc.tile_pool(name="pool", bufs=1) as pool:
            # T row i = in row i (i in [0,26); rows 24,25 = bottom halo)
            T = pool.tile([128, NTM, PADW], f16, name="T", tag="T")
            # T2 row j = in row j-2 (edge tile: 2 top halo + 4 dup rows)
            T2 = pool.tile([128, 6, PADW], f16, name="T2", tag="T2")
            # L row m = in row m-2; [0,2) top halo, [2,26) main,
            # [26,28) bottom halo. Out staging reuses rows [2, 26).
            L = pool.tile([128, RP + 4, WROW], f32, name="L", tag="L")

            e = nc.vector
            s = nc.scalar

            # Setup memzeros run on the DVE (idle during the fill window
            # anyway) so ACT's in-order stream is pure converts — ACT's
            # 4-deep lookahead would otherwise run these ready memzeros
            # ahead of the DMA-blocked first convs.
            def dve_memzero(ap):
                u32 = ap.bitcast(mybir.dt.uint32)
                e.tensor_scalar_mul(u32, u32, 0)

            dve_memzero(T[:, :, 0:PAD])
            dve_memzero(T[:, :, W1:PADW])
            dve_memzero(T2[:, :, 0:PAD])
            dve_memzero(T2[:, :, W1:PADW])
            dve_memzero(L[:, 0:2, :])
            dve_memzero(L[:, RP + 2 : RP + 4, :])
            # dep-free warmup op so the one-time activation-table load
            # (injected before ACT's first Activation) runs at t=0, not
            # when the first conv's input lands
            s.copy(T2[:, 0:1, 0:PAD], T2[:, 0:1, 0:PAD])

            # --- DMA-in: mains in wavefront order; halos mid-stream.
            # conv chunks map to L rows [2,26); the last conv chunk
            # [24,26) reads halo L rows [26,28) and has no main DMA.
            mains = []
            for a, b in conv_c:
                lo, hi = a + 2, min(b + 2, RP + 2)
                if hi > lo:
                    mains.append((lo, hi))

            def main_dma(mi):
                lo, hi = mains[mi]
                if mi < SPLIT_N:
                    # 1-row transfers at the head of the pipe: each conv
                    # row can start as soon as its own row lands
                    for r in range(lo, hi):
                        nc.sync.dma_start(L[:, r : r + 1, :], xap(r - 2, 1))
                else:
                    nc.sync.dma_start(L[:, lo:hi, :], xap(lo - 2, hi - lo))

            def top_halos():
                for b in range(B_PER):
                    p0 = 16 * b + 1
                    nc.sync.dma_start(
                        L[p0 : p0 + 15, 0:2, :], xap(-2, 2, nparts=15, part0=p0)
                    )

            def bottom_halos():
                for b in range(B_PER):
                    p0 = 16 * b
                    nc.sync.dma_start(
                        L[p0 : p0 + 15, RP + 2 : RP + 4, :],
                        xap(RP, 2, nparts=15, part0=p0),
                    )

            for mi in range(len(mains)):
                main_dma(mi)
                if mi == min(TOP_HALO_AFTER, len(mains) - 1):
                    top_halos()
                if mi == min(BOT_HALO_AFTER, len(mains) - 1):
                    bottom_halos()

            # --- compute ops ---
            def conv_in(ci):
                a, b = conv_c[ci]
                if b <= a:
                    return
                if ci < SPLIT_N or ci < DVE_CONV_N:
                    for r in range(a, b):
                        eng = e if ci < DVE_CONV_N else s
                        if eng is e:
                            e.tensor_copy(
                                T[:, r : r + 1, W0:W1], L[:, r + 2 : r + 3, :]
                            )
                        else:
                            s.copy(T[:, r : r + 1, W0:W1], L[:, r + 2 : r + 3, :])
                else:
                    s.copy(T[:, a:b, W0:W1], L[:, a + 2 : b + 2, :])

            def vshift(chunk, shift):
                a, b = chunk
                if b <= a:
                    return
                e.tensor_max(
                    T[:, a:b, W0:W1], T[:, a:b, W0:W1],
                    T[:, a + shift : b + shift, W0:W1],
                )

            def hpasses(tile_, a, b, h3_out=None):
                e.tensor_max(
                    tile_[:, a:b, 0 : PADW - 6], tile_[:, a:b, 0 : PADW - 6],
                    tile_[:, a:b, 6:PADW],
                )
                e.tensor_max(
                    tile_[:, a:b, 0 : PADW - 9], tile_[:, a:b, 0 : PADW - 9],
                    tile_[:, a:b, 3 : PADW - 6],
                )
                # h3_out: write the final pass fp32 straight to the out
                # staging rows (1x rate, but skips the ACT convert hop on
                # the terminal chain)
                out_ap = (
                    tile_[:, a:b, 0:WROW] if h3_out is None else h3_out
                )
                e.tensor_max(
                    out_ap, tile_[:, a:b, 0:WROW],
                    tile_[:, a:b, 3 : 3 + WROW],
                )

            def out_conv(ci):
                # stage at L[a+6:b+6): keeps L[2:6) (edge dup rows) and
                # L[26:28) (bottom halo, read by the last conv chunk at an
                # earlier group) intact until their readers are done.
                a, b = v3_c[ci]
                s.copy(L[:, a + 6 : b + 6, :], T[:, a:b, 0:WROW])

            def out_dma(ci):
                # out-DMAs go through the ACT queue: the SP DMA queue's
                # counting semaphore would otherwise make later input
                # convs falsely wait on earlier output transfers
                a, b = v3_c[ci]
                s.dma_start(yap(a + 2, b - a), L[:, a + 6 : b + 6, :])

            def edge_v2_copy():
                # T2 rows 2,3 <- main V2 rows 0,1 (win3 over in 0..2/1..3),
                # DVE TensorCopy f16 all-SBUF at 4x, between V2 c0 and V3 c0.
                # Supersets of the nominal edge windows are harmless for max.
                e.tensor_copy(T2[:, 2:4, W0:W1], T[:, 0:2, W0:W1])

            def edge_ops():
                s.copy(T2[:, 0:2, W0:W1], L[:, 0:2, :])
                e.tensor_max(
                    T2[:, 0:2, W0:W1], T2[:, 0:2, W0:W1], T2[:, 1:3, W0:W1]
                )
                e.tensor_max(
                    T2[:, 0:2, W0:W1], T2[:, 0:2, W0:W1], T2[:, 1:3, W0:W1]
                )
                e.tensor_max(
                    T2[:, 0:2, W0:W1], T2[:, 0:2, W0:W1], T2[:, 2:4, W0:W1]
                )
                hpasses(T2, 0, 2)
                s.copy(L[:, 2:4, :], T2[:, 0:2, 0:WROW])
                s.dma_start(yap(0, 2), L[:, 2:4, :])

            # --- wavefront emission ---
            # group k: conv ck, v1 ck, v2 c(k-1), v3 c(k-2), H+out c(k-2)
            conv_in(0)
            v1c = v2c = v3c = 0
            for k in range(n + 2):
                if k + 1 < n:
                    conv_in(k + 1)
                if v1c <= k and v1c < len(v1_c):
                    vshift(v1_c[v1c], 1)
                    v1c += 1
                if v2c <= k - 1 and v2c < len(v2_c):
                    vshift(v2_c[v2c], 1)
                    v2c += 1
                    if v2c == 1:
                        assert v2_e[0] >= 2
                        edge_v2_copy()
                if v3c <= k - 2 and v3c < len(v3_c):
                    a, b = v3_c[v3c]
                    if b > a:
                        vshift(v3_c[v3c], 2)
                        if v3c == len(v3_c) - 1 and TAIL_COLSPLIT:
                            # final row: H1/H2 whole, then column-split the
                            # fused H3 so each half's out-DMA overlaps the
                            # other half's compute
                            e.tensor_max(
                                T[:, a:b, 0 : PADW - 6],
                                T[:, a:b, 0 : PADW - 6], T[:, a:b, 6:PADW],
                            )
                            e.tensor_max(
                                T[:, a:b, 0 : PADW - 9],
                                T[:, a:b, 0 : PADW - 9],
                                T[:, a:b, 3 : PADW - 6],
                            )
                            hw_ = WROW // 2
                            for c0, c1 in ((0, hw_), (hw_, WROW)):
                                e.tensor_max(
                                    L[:, a + 6 : b + 6, c0:c1],
                                    T[:, a:b, c0:c1],
                                    T[:, a:b, c0 + 3 : c1 + 3],
                                )
                                s.dma_start(
                                    AP(
                                        y,
                                        (a + 2) * WROW + c0,
                                        [
                                            [RP * WROW, 128],
                                            [WROW, b - a],
                                            [1, c1 - c0],
                                        ],
                                    ),
                                    L[:, a + 6 : b + 6, c0:c1],
                                )
                        elif v3c >= len(v3_c) - FUSE_LAST:
                            hpasses(T, a, b, h3_out=L[:, a + 6 : b + 6, :])
                            out_dma(v3c)
                        else:
                            hpasses(T, a, b)
                            out_conv(v3c)
                            out_dma(v3c)
                    v3c += 1
                if k == edge_after:
                    edge_ops()

    nc.compile()
    return nc


def _get_nc():
    if "nc" not in _CACHE:
        _CACHE["nc"] = _build_nc()
    return _CACHE["nc"]


def _run(images, trace=False):
    _ensure_path()
    from concourse import bass_utils

    images = np.ascontiguousarray(np.asarray(images, dtype=np.float32))
    assert images.shape == (N_CORES * B_PER, H, W, C), images.shape
    nc = _get_nc()
    per_core = images.reshape(N_CORES, ROWS, WROW)
    in_maps = [{"x": np.ascontiguousarray(per_core[i])} for i in range(N_CORES)]
    res = bass_utils.run_bass_kernel_spmd(
        nc, in_maps, core_ids=list(range(N_CORES)), trace=trace
    )
    out = np.concatenate([res.results[i]["y"] for i in range(N_CORES)], axis=0)
    out = out.reshape(N_CORES * B_PER, H, W, C)[..., None]
    return out, res


def kernel(images, k=None):
    out, _ = _run(images, trace=False)
    return out
